# revision 41
# baseline (speedup 1.0000x reference)
"""Trainium2 Bass kernel for nn_Attn_88725434401526 (sparse_attention).

Reference computation:
    delta[b,l,m] = sum_d self_delta[b,m,l,d]
    P[b,l,m]     = emb_table[1+l] . self_attn[b,m]
    out[b,l]     = sum_m P[b,l,m] * delta[b,l,m] * value_w[0,m]

Shapes: B=16, MAX_LEN(m)=100, LOC_MAX(l)=20000, EMB=256, D=2.
Output: [16, 20000] float32.

Strategy (8 NeuronCores, loc_max sharded -> 2500 candidates per core):
  - Host staging: delta pre-summed over d and cast to fp16 -> the
    per-core stream drops from 32MB f32 to 8MB fp16, taking DMA off
    the critical path. emb/attn/value_w also staged fp16 (as before).
  - (b,m) = 1600 rows in 13 chunks of 128 partitions.
  - P2[(b,m), l] = attn . emb via PE matmul in fp16 (K=EMB as 2x128),
    fp32 PSUM, 5 l-tiles of 512 per chunk.  ~65k PE cycles.
  - ACT copies P2 PSUM->SBUF as fp16; DVE multiplies by delta (fp16
    2x mode).
  - weighted reduction over m: second matmul with a block matrix
    carrying value_w (stationary [128,16] per chunk), 4x column-tiled
    across PE col-groups (out partition groups 0/32/64/96) so the four
    l-quarters stream concurrently; accumulated over the 13 chunks in
    PSUM ([16,512]+[16,113] per group).

kernel(**inputs) takes the FULL unsharded inputs (numpy, keyed as in
setup_inputs()) and returns the FULL [16, 20000] float32 output.
"""
import sys

if "/opt/trn_rl_repo" not in sys.path:
    sys.path.insert(0, "/opt/trn_rl_repo")

import numpy as np
import ml_dtypes
import concourse.bass as bass
import concourse.mybir as mybir
from concourse import tile
from concourse.bass_utils import run_bass_kernel_spmd

FP32 = mybir.dt.float32
FP16 = mybir.dt.float16

B = 16
M = 100
LOC = 20000
EMB = 256
NCORES = 8
LCORE = LOC // NCORES          # 2500 candidates per core
G = B * M                      # 1600 (b,m) rows
P = 128
NCHUNK = (G + P - 1) // P      # 13 row chunks; last has 64 rows
LSTEP = 512
LTILE = 1024                   # P2 PSUM tile width (2 banks)
LOFFS = [0, 1024, 2048]
LWIDTH = [1024, 1024, LCORE - 2048]           # [1024, 1024, 452]
LQ = LCORE // 4                # 625: per-col-group l quarter
LQA = 512                      # quarter split: 512 + 113 (PSUM bank cap)
LQB = LQ - LQA


def _split_multi_waits(nc, maxw=1):
    """walrus codegen rejects >1 semaphore wait per instruction; split
    extra waits onto preceding NOPs on the same engine."""
    for fn in nc.m.functions:
        for bb in fn.blocks:
            newl = []
            for inst in bb.instructions:
                si = inst.sync_info
                if si is not None and si.on_wait and len(si.on_wait) > maxw:
                    waits = list(si.on_wait)
                    head, tail = waits[:-maxw], waits[-maxw:]
                    for i0 in range(0, len(head), maxw):
                        newl.append(
                            mybir.InstNoOp(
                                name=f"I-waitsplit-{nc.next_id()}",
                                engine=inst.engine,
                                sync_info=mybir.SyncInfo(
                                    on_wait=list(head[i0 : i0 + maxw]),
                                    on_update=[],
                                ),
                            )
                        )
                    inst.sync_info = mybir.SyncInfo(
                        on_wait=list(tail), on_update=list(si.on_update)
                    )
                newl.append(inst)
            bb.instructions = newl


def build_nc():
    nc = bass.Bass()
    sd = nc.declare_dram_parameter("sd", [G, LCORE], FP16, isOutput=False)
    embT = nc.declare_dram_parameter("embT", [2, P, LCORE], FP16, isOutput=False)
    attnT = nc.declare_dram_parameter("attnT", [2, P, G], FP16, isOutput=False)
    wseg = nc.declare_dram_parameter("wseg", [P, NCHUNK * B], FP16, isOutput=False)
    out = nc.declare_dram_parameter("out", [B, LCORE], FP32, isOutput=True)

    with tile.TileContext(nc) as tc:
        with (
            tc.tile_pool(name="const", bufs=1) as cpool,
            tc.tile_pool(name="sdp", bufs=4) as sdpool,
            tc.tile_pool(name="p2sbp", bufs=2) as p2sbpool,
            tc.tile_pool(name="prodp", bufs=3) as prodpool,
            tc.tile_pool(name="outp", bufs=1) as outpool,
            tc.tile_pool(name="ps", bufs=2, space="PSUM") as pspool,
            tc.tile_pool(name="pss", bufs=2, space="PSUM") as psspool,
            tc.tile_pool(name="pso", bufs=1, space="PSUM") as psopool,
        ):
            # one SBUF tile per staging DMA so consumers wait on exactly
            # the transfer they need (whole-tile write tracking).
            # chunk-0-critical slices get their own small tiles + early
            # DMAs on the ACT HWDGE queue; the bulk goes via the idle
            # GPSIMD SWDGE queue.
            attnT_c0 = [
                cpool.tile([P, P], FP16, name=f"attnTc0_{k}") for k in range(2)
            ]
            attnT_r = [
                cpool.tile([P, G - P], FP16, name=f"attnTr{k}")
                for k in range(2)
            ]
            embT_l = [
                [
                    cpool.tile([P, lw], FP16, name=f"embT{k}_{li}")
                    for li, lw in enumerate(LWIDTH)
                ]
                for k in range(2)
            ]
            wseg_t = cpool.tile([P, NCHUNK * B], FP16)
            warm_t = cpool.tile([P, LSTEP], FP16)
            sd_tiles = {}

            def attn_ap(k, g0, g1):
                # stationary slice [P, g0:g1] from the split staging tiles
                if g1 <= P:
                    return attnT_c0[k][:, g0:g1]
                return attnT_r[k][:, g0 - P : g1 - P]

            def emb_ap(k, l0, lw):
                for li, (lo, lwid) in enumerate(zip(LOFFS, LWIDTH)):
                    if lo <= l0 < lo + lwid:
                        return embT_l[k][li][:, l0 - lo : l0 - lo + lw]
                raise AssertionError(l0)

            def sd_slice(p):
                g0 = p * P
                rows = min(P, G - g0)
                t = sdpool.tile([P, LCORE], FP16, tag="sd")
                nc.sync.dma_start(t[:rows, :], sd[g0 : g0 + rows, :])
                sd_tiles[p] = t

            # HAM warm-up while staging streams in (reduce accumulator
            # bank doubles as scratch; first real reduce MM has start=True)
            nc.gpsimd.memset(warm_t[:], 0.0)

            # critical staging (chunk0 k0/k1) on the ACT queue
            nc.scalar.dma_start(attnT_c0[0][:], attnT[0, :, :P])
            nc.scalar.dma_start(embT_l[0][0][:], embT[0, :, : LWIDTH[0]])
            nc.scalar.dma_start(attnT_c0[1][:], attnT[1, :, :P])
            nc.scalar.dma_start(embT_l[1][0][:], embT[1, :, : LWIDTH[0]])
            nc.scalar.dma_start(wseg_t[:], wseg[:, :])
            # bulk staging on the sync queue, ahead of the sd stream
            for li in (1, 2):
                for k in range(2):
                    nc.sync.dma_start(
                        embT_l[k][li][:],
                        embT[k, :, LOFFS[li] : LOFFS[li] + LWIDTH[li]],
                    )
            nc.sync.dma_start(attnT_r[0][:], attnT[0, :, P:])
            nc.sync.dma_start(attnT_r[1][:], attnT[1, :, P:])
            sd_slice(0)
            sd_slice(1)

            # reduction accumulators: col-group q uses out partitions
            # [32q, 32q+16) -> tile_position (0, 32q) auto-derived.
            psout_a = psopool.tile([P, LQA], FP32)
            psout_b = psopool.tile([P, P], FP32)
            out_sb = outpool.tile([P, LQ], FP32)

            for _ in range(25):
                nc.tensor.matmul(
                    psout_a[:, :],
                    warm_t[:, :P],
                    warm_t[:, :LQA],
                    start=True,
                    stop=True,
                    skip_group_check=True,
                )

            p2sb_tiles = {}
            prod_tiles = {}

            def emit_front(p):
                g0 = p * P
                rows = min(P, G - g0)
                prod = prodpool.tile([P, LCORE], FP16, tag="prod")
                prod_tiles[p] = prod
                if p < NCHUNK - 1:
                    p2sb = p2sbpool.tile([P, LCORE], FP16, tag="p2sb")
                    p2sb_tiles[p] = p2sb
                for li, (l0, lw) in enumerate(zip(LOFFS, LWIDTH)):
                    big = li < 2
                    if big:
                        ps = pspool.tile([P, LTILE], FP32, name="psb")
                    else:
                        ps = psspool.tile([P, LSTEP], FP32, name="pss")
                    # PSUM-bank-sized matmuls (<=512 fp32 columns each)
                    for k in range(2):
                        for c0 in range(0, lw, LSTEP):
                            cw = min(LSTEP, lw - c0)
                            nc.tensor.matmul(
                                ps[:rows, c0 : c0 + cw],
                                attn_ap(k, g0, g0 + rows),
                                emb_ap(k, l0 + c0, cw),
                                start=(k == 0),
                                stop=(k == 1),
                            )
                    if p == NCHUNK - 1:
                        # last chunk: fused PSUM*delta on DVE shortens
                        # the drain into the final reduce matmuls
                        nc.vector.tensor_tensor(
                            prod[:rows, l0 : l0 + lw],
                            ps[:rows, :lw],
                            sd_tiles[p][:rows, l0 : l0 + lw],
                            mybir.AluOpType.mult,
                        )
                        continue
                    if big:
                        # ACT releases the 2-bank tiles (one big op)
                        nc.scalar.copy(
                            p2sb[:rows, l0 : l0 + lw], ps[:rows, :lw]
                        )
                    else:
                        # DVE releases the 452 tail
                        nc.vector.tensor_scalar_mul(
                            p2sb[:rows, l0 : l0 + lw], ps[:rows, :lw], 1.0
                        )
                    nc.vector.tensor_tensor(
                        prod[:rows, l0 : l0 + lw],
                        p2sb[:rows, l0 : l0 + lw],
                        sd_tiles[p][:rows, l0 : l0 + lw],
                        mybir.AluOpType.mult,
                    )

            def emit_back(p):
                g0 = p * P
                rows = min(P, G - g0)
                prod = prod_tiles.pop(p)
                p2sb_tiles.pop(p, None)
                w = wseg_t[:rows, p * B : (p + 1) * B]
                for q in range(4):
                    nc.tensor.matmul(
                        psout_a[32 * q : 32 * q + B, :],
                        w,
                        prod[:rows, LQ * q : LQ * q + LQA],
                        start=(p == 0),
                        stop=(p == NCHUNK - 1),
                        skip_group_check=True,
                        tile_position=(0, 32 * q),
                    )
                for q in range(4):
                    nc.tensor.matmul(
                        psout_b[32 * q : 32 * q + B, :LQB],
                        w,
                        prod[:rows, LQ * q + LQA : LQ * (q + 1)],
                        start=(p == 0),
                        stop=(p == NCHUNK - 1),
                        skip_group_check=True,
                        tile_position=(0, 32 * q),
                    )

            for p in range(NCHUNK):
                if p + 2 < NCHUNK:
                    sd_slice(p + 2)
                emit_front(p)
                if p > 1:
                    emit_back(p - 2)
            emit_back(NCHUNK - 2)
            emit_back(NCHUNK - 1)

            # final: PSUM -> SBUF -> DRAM, split across ACT + DVE
            for q in range(4):
                ga, gb = 32 * q, 32 * q + B
                if q % 2 == 0:
                    nc.scalar.copy(out_sb[ga:gb, :LQA], psout_a[ga:gb, :])
                    nc.vector.tensor_scalar_mul(
                        out_sb[ga:gb, LQA:LQ], psout_b[ga:gb, :LQB], 1.0
                    )
                    nc.scalar.dma_start(
                        out[:, LQ * q : LQ * (q + 1)], out_sb[ga:gb, :]
                    )
                else:
                    nc.vector.tensor_scalar_mul(
                        out_sb[ga:gb, :LQA], psout_a[ga:gb, :], 1.0
                    )
                    nc.scalar.copy(out_sb[ga:gb, LQA:LQ], psout_b[ga:gb, :LQB])
                    nc.sync.dma_start(
                        out[:, LQ * q : LQ * (q + 1)], out_sb[ga:gb, :]
                    )

    _split_multi_waits(nc)
    return nc


_NC_CACHE = None


def _get_nc():
    global _NC_CACHE
    if _NC_CACHE is None:
        _NC_CACHE = build_nc()
    return _NC_CACHE


def make_in_maps(self_attn, self_delta, emb_table, value_w):
    self_attn = np.ascontiguousarray(self_attn, dtype=np.float32)
    emb_table = np.ascontiguousarray(emb_table, dtype=np.float32)
    value_w = np.ascontiguousarray(value_w, dtype=np.float32)
    f16 = ml_dtypes.float16 if hasattr(ml_dtypes, "float16") else np.float16

    # host-side d-reduction: [B, M, LOC, 2] -> [G, LOC] fp16
    sd32 = np.asarray(self_delta, dtype=np.float32)
    delta = (sd32[..., 0] + sd32[..., 1]).reshape(G, LOC)

    # attnT: [2, 128, 1600] = self_attn reshaped [(b,m), e], transposed
    attnT = (
        np.ascontiguousarray(self_attn.reshape(G, EMB).T)
        .reshape(2, P, G)
        .astype(f16)
    )

    # wseg block matrix [128, 13*16]; wseg[r, p*16+b] = w[m] for g=128p+r
    w = value_w[0]
    wsegm = np.zeros((NCHUNK, P, B), np.float32)
    g = np.arange(G)
    wsegm[g // P, g % P, g // M] = w[g % M]
    wsegm = np.ascontiguousarray(
        wsegm.transpose(1, 0, 2).reshape(P, NCHUNK * B)
    ).astype(f16)

    embT_all = np.ascontiguousarray(emb_table[1 : LOC + 1].T)  # [256, 20000]

    in_maps = []
    for c in range(NCORES):
        l0 = c * LCORE
        sd_c = np.ascontiguousarray(delta[:, l0 : l0 + LCORE].astype(f16))
        embT_c = (
            np.ascontiguousarray(embT_all[:, l0 : l0 + LCORE])
            .reshape(2, P, LCORE)
            .astype(f16)
        )
        in_maps.append(
            {"sd": sd_c, "embT": embT_c, "attnT": attnT, "wseg": wsegm}
        )
    return in_maps


def kernel(self_attn, self_delta, traj_len, emb_table, value_w, **_ignored):
    nc = _get_nc()
    in_maps = make_in_maps(self_attn, self_delta, emb_table, value_w)
    res = run_bass_kernel_spmd(nc, in_maps, list(range(NCORES)))
    return np.concatenate(
        [np.asarray(res.results[c]["out"]) for c in range(NCORES)], axis=1
    )
